# revision 7
# baseline (speedup 1.0000x reference)
"""BitLinear (absmean-ternary quantized linear) Trainium2 kernel.

Computes: out = x @ ternarize(weight).T + bias
  where ternarize(w) = sign(w) * (|w| >= 0.7 * mean(|w|)), all in fp32.

Sharding: tensor-parallel over out_features across 8 NeuronCores
(column-parallel): weight/bias sharded, x replicated, outputs concatenated.

Device strategy per core:
  - host precomputes threshold thr = 0.7*mean(|w|) in fp32 (bitwise identical
    to XLA:CPU's fp32 mean on this reduction), transposes x and w so all
    device DMAs are natural-layout.
  - quantize w-shard tiles on VectorE into an SBUF-resident ternary weight
    [128, 4096/128, O_half] (exact in bf16/f32r).
  - matmul: x tile [128k x 128t] stationary, ternary w [128k x 512o] moving,
    accumulate K=4096 in PSUM fp32, add bias on eviction.

Matmul dtype modes (BL_MM_DT env): "f32r" (fp32-storage reduced-precision PE
mode, ~1.5e-4 rel err) or "bf16" (x rounded to bf16 on host, ~1.7e-3 rel err).
Both stream 1 row/cycle on the PE.
"""

import os

import numpy as np

import concourse.bass as bass  # noqa: F401  (bass must be imported before tile)
import concourse.mybir as mybir
import concourse.tile as tile
from concourse import bacc
from concourse.bass_utils import run_bass_kernel_spmd

TOKENS = 8192
IN_F = 4096
OUT_F = 16384
NCORES = 8
O_SHARD = OUT_F // NCORES  # 2048
P = 128
KO = IN_F // P  # 32 k-slabs of 128
MT = TOKENS // P  # 64 token tiles
NFREE = 512  # psum free width (one bank)

MODE = os.environ.get("BL_MM_DT", "f32r")

_compiled = {}


def build(mode=MODE, repeat=1):
    """Build + compile the SPMD program. `repeat` re-runs the matmul phase
    (timing amplification only; results identical)."""
    is_bf16 = mode == "bf16"
    n_halves = 1 if is_bf16 else 2
    O_HALF = O_SHARD // n_halves
    mm_dt = mybir.dt.bfloat16 if is_bf16 else mybir.dt.float32r
    x_dt = mybir.dt.bfloat16 if is_bf16 else mybir.dt.float32r

    nc = bacc.Bacc(None, target_bir_lowering=False, debug=False, num_devices=NCORES)

    xT = nc.dram_tensor("xT", [IN_F, TOKENS], x_dt, kind="ExternalInput")
    wT = nc.dram_tensor("wT", [IN_F, O_SHARD], mybir.dt.float32, kind="ExternalInput")
    bias_d = nc.dram_tensor("bias", [O_SHARD], mybir.dt.float32, kind="ExternalInput")
    thr_d = nc.dram_tensor("thr", [1], mybir.dt.float32, kind="ExternalInput")
    out = nc.dram_tensor(
        "out", [TOKENS, O_SHARD], mybir.dt.float32, kind="ExternalOutput"
    )

    xT_v = xT.ap().rearrange("(ko p) t -> p ko t", p=P)
    wT_v = wT.ap().rearrange("(ko p) o -> p ko o", p=P)
    out_v = out.ap().rearrange("(mo p) o -> p mo o", p=P)

    with tile.TileContext(nc) as tc:
        with (
            tc.tile_pool(name="const", bufs=1) as const,
            tc.tile_pool(name="wqp", bufs=1) as wqp,
            tc.tile_pool(name="stage", bufs=2) as stage,
            tc.tile_pool(name="xp", bufs=2) as xp,
            tc.tile_pool(name="outp", bufs=4) as outp,
            tc.tile_pool(name="psum", bufs=4, space="PSUM") as psum,
        ):
            thr_both = const.tile([P, 2], mybir.dt.float32)
            thr_sb = thr_both[:, 0:1]
            negthr_sb = thr_both[:, 1:2]
            nc.sync.dma_start(thr_sb, thr_d.ap().to_broadcast((P, 1)))
            nc.vector.tensor_scalar_mul(negthr_sb, thr_sb, -1.0)
            bias_sb = const.tile([P, O_SHARD], mybir.dt.float32)
            nc.sync.dma_start(
                bias_sb[:], bias_d.ap()[None, :].to_broadcast((P, O_SHARD))
            )

            for h in range(n_halves):
                o_base = h * O_HALF
                # ternarize this half of the weight shard into resident SBUF
                wq = wqp.tile([P, KO, O_HALF], mm_dt, tag="wq")
                for ko in range(KO):
                    st = stage.tile([P, O_HALF], mybir.dt.float32, tag="wstage")
                    nc.sync.dma_start(
                        st[:], wT_v[:, ko, o_base : o_base + O_HALF]
                    )
                    tmp = stage.tile([P, O_HALF], mybir.dt.float32, tag="wtmp")
                    # tmp = (w > -thr) - 1  ->  {-1, 0}
                    nc.vector.tensor_scalar(
                        tmp[:],
                        st[:],
                        negthr_sb[:],
                        -1.0,
                        op0=mybir.AluOpType.is_gt,
                        op1=mybir.AluOpType.add,
                    )
                    # wq = (w >= thr) + tmp  ->  {-1, 0, 1}
                    nc.vector.scalar_tensor_tensor(
                        wq[:, ko, :],
                        st[:],
                        thr_sb[:],
                        tmp[:],
                        op0=mybir.AluOpType.is_ge,
                        op1=mybir.AluOpType.add,
                    )

                for _rep in range(repeat):
                    for m in range(MT):
                        xt = xp.tile([P, KO, P], x_dt, tag="xt")
                        nc.sync.dma_start(xt[:], xT_v[:, :, m * P : (m + 1) * P])
                        for n in range(O_HALF // NFREE):
                            ps = psum.tile([P, NFREE], mybir.dt.float32)
                            for k in range(KO):
                                nc.tensor.matmul(
                                    ps[:],
                                    lhsT=xt[:, k, :],
                                    rhs=wq[:, k, n * NFREE : (n + 1) * NFREE],
                                    start=(k == 0),
                                    stop=(k == KO - 1),
                                )
                            ot = outp.tile([P, NFREE], mybir.dt.float32, tag="ot")
                            o0 = o_base + n * NFREE
                            nc.vector.tensor_add(
                                out=ot[:],
                                in0=ps[:],
                                in1=bias_sb[:, o0 : o0 + NFREE],
                            )
                            nc.sync.dma_start(out_v[:, m, o0 : o0 + NFREE], ot[:])

    nc.compile()
    return nc


def _get_compiled(mode):
    if mode not in _compiled:
        _compiled[mode] = build(mode)
    return _compiled[mode]


def kernel(x, weight, bias):
    x = np.ascontiguousarray(np.asarray(x, dtype=np.float32))
    weight = np.ascontiguousarray(np.asarray(weight, dtype=np.float32))
    bias = np.ascontiguousarray(np.asarray(bias, dtype=np.float32))

    # fp32 absmean threshold; np.mean's pairwise fp32 reduction is bitwise
    # identical to XLA:CPU's fp32 mean here.
    scale = np.float32(np.mean(np.abs(weight)))
    thr = np.full((1,), np.float32(scale * np.float32(0.7)), dtype=np.float32)

    xT = np.ascontiguousarray(x.T)
    if MODE == "bf16":
        import ml_dtypes

        xT = xT.astype(ml_dtypes.bfloat16)
    wT = np.ascontiguousarray(weight.T)  # [IN_F, OUT_F]

    in_maps = []
    for c in range(NCORES):
        sl = slice(c * O_SHARD, (c + 1) * O_SHARD)
        in_maps.append(
            {
                "xT": xT,
                "wT": np.ascontiguousarray(wT[:, sl]),
                "bias": np.ascontiguousarray(bias[sl]),
                "thr": thr,
            }
        )

    nc = _get_compiled(MODE)
    res = run_bass_kernel_spmd(nc, in_maps, list(range(NCORES)))
    return np.concatenate(
        [res.results[c]["out"] for c in range(NCORES)], axis=1
    ).astype(np.float32, copy=False)
